# revision 43
# baseline (speedup 1.0000x reference)
"""Fused GEMM + bias + residual + AvgPool2d(2) + global-mean normalize, 8-core SPMD.

Reference computation (B=8192, IN_F=1024, OUT_F=4096, S=64, K=2):
    out_lin = x @ W.T + bias + y                  # (B, 4096)
    pooled  = avgpool2x2(out_lin.reshape(B,64,64))# (B, 32, 32)
    out     = pooled / pooled.mean()              # (B, 1, 32, 32)

Algebraic folds (all exact):
  * The 2x2 avg-pool folds into weight/bias/residual:
        pooled_raw[b, m] = x[b] . Wsum[m] + bias_sum[m] + y_sum[b, m]
    where m = 32*i + j pools OUT_F rows {128i+2j, 128i+2j+1, 128i+64+2j,
    128i+64+2j+1}.  GEMM N-dim shrinks 4096 -> 1024.
  * The 1/4 pool factor cancels against the global mean:
        out = pooled_raw * (B*1024 / sum_global(pooled_raw))
  * The global sum decomposes over raw inputs:
        local_sum = xsum . wcolsum + BL * bias_tot + ytot

Distribution (this version):
  * Batch B split 8 ways (1024 rows/core); bias replicated.
  * W-pooling is SHARDED: core c receives only W rows [512c, 512c+512)
    (pooled features m in [128c, 128c+128)), pools + transposes its
    128-feature slice to bf16 [k, m] layout, then one 256 KiB/rank
    AllGather replicates the full pooled weight (2 MiB) to every core.
    Cuts per-core W HBM traffic 16 MiB -> ~6 MiB.
  * The global mean uses a [128]-vector AllReduce (per-partition partial
    sums); the 128-way partition reduction happens AFTER the collective
    on the readback (cheap), so no PE matmul sits on the trigger path.
    The trigger fires ~2us after the last y tile is pooled.

Engine plan:
  * rings: sync carries W half 0 + even x/y tiles + stores; scalar
    carries W half 1 + bias + odd x/y tiles + stores.
  * gpsimd: SWDGE store of the W contribution, AllGather (blocks its
    queue over the otherwise-idle early window), SWDGE readback of the
    gathered weight, odd-bt y pooling, the scalar AllReduce, odd
    normalizes.
  * DVE: W/bias pooling, even-bt y pooling, all reductions, GEMM
    epilogue adds, even normalizes.
  * ACT: PSUM->SBUF copies (W perm-fix + x transpose casts).
  * PE: W transposes, x transposes, GEMM.
"""

import numpy as np

import concourse.bass as bass
import concourse.mybir as mybir
import concourse.tile as tile
from concourse import bacc, bass_isa
from concourse.bass import ts
from concourse.bass_utils import run_bass_kernel_spmd
from concourse.masks import make_identity

N_CORES = 8
B = 8192
BL = B // N_CORES          # 1024 batch rows per core
KF = 1024                  # IN_F (contraction)
NF = 4096                  # OUT_F
M = 1024                   # pooled features (32*32)
WS = NF // N_CORES         # 512 W rows per core
TOT = float(B * M)         # elements in the global mean
F32 = mybir.dt.float32
F16 = mybir.dt.float16
BF16 = mybir.dt.bfloat16
ADD = mybir.AluOpType.add
MULT = mybir.AluOpType.mult

_CACHE = {}


def build_nc():
    nc = bacc.Bacc("TRN2", target_bir_lowering=False, debug=False,
                   num_devices=N_CORES)
    x = nc.dram_tensor("x", [BL, KF], F32, kind="ExternalInput").ap()
    y = nc.dram_tensor("y", [BL, NF], F32, kind="ExternalInput").ap()
    w = nc.dram_tensor("w", [WS, KF], F32, kind="ExternalInput").ap()
    b = nc.dram_tensor("b", [1, NF], F32, kind="ExternalInput").ap()
    out = nc.dram_tensor("out", [BL, M], F16, kind="ExternalOutput").ap()

    # Collective buffers.  Contribution = this core's pooled+transposed
    # weight slice [k_in, kb, ml]; gather concatenates rank-major.
    cc_wt_in = nc.dram_tensor("cc_wt_in", [128, 8, 128], BF16,
                              kind="Internal").ap()
    cc_wt_out = nc.dram_tensor("cc_wt_out", [8, 128, 8, 128], BF16,
                               kind="Internal", addr_space="Shared").ap()
    # stats AllReduce payload [128 k, 17]: cols 0:8 = per-core wcol
    # partials, cols 8:16 = per-core xsum, col 16 = y+bias partials.
    # gsum = sum_k wcol_g[k]*xsum_g[k] + sum(col16) after the reduce;
    # the partition reduction runs post-collective on gpsimd (axis C).
    cc_s_in = nc.dram_tensor("cc_s_in", [128, 17], F32, kind="Internal").ap()
    cc_s_out = nc.dram_tensor("cc_s_out", [128, 17], F32,
                              kind="Internal", addr_space="Shared").ap()

    # This core's W rows n = 128a + 64r + 2j + s (a=4, r=2, j=32, s=2);
    # local pooled feature ml = 32a + j sums the 4 taps (r, s).
    # Row-pair index np = 64a + 32r + j; pairs (2j, 2j+1) are contiguous.
    w_pairs = w.rearrange("(n s) k -> n (s k)", s=2)          # [256, 2048]
    wv = w_pairs.rearrange("(a r j) kk -> r j a kk", a=4, r=2, j=32)

    ring = [nc.sync, nc.scalar]

    with tile.TileContext(nc) as tc:
        with (
            tc.tile_pool(name="consts", bufs=1) as consts,
            tc.tile_pool(name="wload", bufs=1) as wload,
            tc.tile_pool(name="bloadp", bufs=1) as bloadp,
            tc.tile_pool(name="wsump", bufs=1) as wsump,
            tc.tile_pool(name="wtp", bufs=1) as wtp,
            tc.tile_pool(name="wallp", bufs=1) as wallp,
            tc.tile_pool(name="xload", bufs=5) as xload,
            tc.tile_pool(name="xtp", bufs=8) as xtp,
            tc.tile_pool(name="yload", bufs=5) as yload,
            tc.tile_pool(name="yup", bufs=3) as yup,
            tc.tile_pool(name="ysump", bufs=1) as ysump,
            tc.tile_pool(name="statsp", bufs=1) as statsp,
            tc.tile_pool(name="outp", bufs=3) as outp,
            tc.tile_pool(name="psA", bufs=4, space="PSUM") as psA,
            tc.tile_pool(name="psT", bufs=3, space="PSUM") as psT,
        ):
            # ---- constants ----
            ident = consts.tile([128, 128], BF16)
            make_identity(nc, ident)
            ident_f = consts.tile([128, 128], F32)
            make_identity(nc, ident_f)
            ones_row = consts.tile([1, 128], BF16)
            nc.vector.memset(ones_row, 1.0)
            tot_pp = consts.tile([128, 1], F32)
            nc.vector.memset(tot_pp, TOT)

            # ---- W slice: 2 x 1 MiB loads, pool 512 -> 128 rows ----
            wl = wload.tile([128, 2, 2048], F32)
            nc.sync.dma_start(out=wl[:, 0, :], in_=wv[0])
            nc.scalar.dma_start(out=wl[:, 1, :], in_=wv[1])

            # pool the W slice on DVE FIRST (shortest path to the
            # AllGather doorbell): partition p = 4j + a
            wlv = wl.rearrange("p r (s k) -> p r s k", s=2)
            t1 = wsump.tile([128, KF], F32)
            nc.vector.tensor_add(t1, wlv[:, 0, 0], wlv[:, 0, 1])
            t2 = wsump.tile([128, KF], F32)
            nc.vector.tensor_add(t2, wlv[:, 1, 0], wlv[:, 1, 1])
            wsum = wsump.tile([128, KF], BF16)
            nc.vector.tensor_add(wsum, t1, t2)

            # transpose to [k_in, kb, ml] (fix p = 4j + a -> ml = 32a + j
            # inside the PSUM->SBUF copy on ACT)
            wt_local = wtp.tile([128, 8, 128], BF16)
            for kb in range(8):
                pt = psT.tile([128, 128], BF16, tag="pt", name=f"ptw{kb}")
                nc.tensor.transpose(pt, wsum[:, ts(kb, 128)], ident)
                nc.scalar.copy(
                    out=wt_local[:, kb, :].rearrange("k (a j) -> k j a", a=4),
                    in_=pt.rearrange("k (j a) -> k j a", a=4))

            # this core's wcol partial (the AllReduce sums it globally)
            ar_in = statsp.tile([128, 17], F32)
            nc.vector.reduce_sum(
                out=ar_in[:, 0:8].unsqueeze(2), in_=wt_local,
                axis=mybir.AxisListType.X)

            # contribution store + AllGather + readback, all on gpsimd so
            # no HWDGE ring ever stalls on collective latency
            nc.gpsimd.dma_start(out=cc_wt_in, in_=wt_local)
            # NOTE: the AllGather itself is emitted inside the streaming
            # loop (after bt 3's pooling).  ncfw can't serve it before its
            # ~45-65us exec-start init anyway, and this way gpsimd pools
            # bt 1/3 before blocking on the collective.
            # SBUF layout [k_in, kb, c, ml]: GEMM rhs slice [:, kb, 4mh:4mh+4, :]
            # is 512 contiguous m-columns in natural order (m = 128c + ml).
            # The readback DMAs are emitted AFTER the y loop (rings are idle
            # by then, and nothing else queues behind them), one contiguous
            # 256 KiB DMA per source rank, split across both rings.
            wt_all = wallp.tile([128, 8, 8, 128], BF16)

            # ---- bias: load + pool 4096 -> 1024 (three 1-partition adds)
            bload = bloadp.tile([1, NF], F32)
            nc.scalar.dma_start(out=bload, in_=b)
            blv = bload.rearrange("o (i r j s) -> o i r j s", r=2, j=32, s=2)
            bsum = consts.tile([1, 32, 32], F32)
            nc.vector.tensor_add(bsum, blv[:, :, 0, :, 0], blv[:, :, 0, :, 1])
            nc.vector.tensor_add(bsum, bsum, blv[:, :, 1, :, 0])
            nc.vector.tensor_add(bsum, bsum, blv[:, :, 1, :, 1])
            bsum_bf = consts.tile([1, M], BF16)
            nc.vector.tensor_copy(out=bsum_bf,
                                  in_=bsum.rearrange("o i j -> o (i j)"))
            btot = consts.tile([1, 1], F32)
            nc.vector.reduce_sum(out=btot,
                                 in_=bsum.rearrange("o i j -> o (i j)"),
                                 axis=mybir.AxisListType.X)
            btot_s = consts.tile([1, 1], F32)
            nc.scalar.mul(btot_s, btot, float(BL))

            # ---- stream x + y per b-tile: transpose x on PE (copies
            # alternate ACT/DVE), pool y (DVE evens / gpsimd odds) ----
            psums_all = statsp.tile([128, 8], F32)
            xsum_acc = ar_in[:, 8:16]
            xts = []
            ys_tiles = {}
            for bt in range(8):
                xf = xload.tile([128, KF], F32, tag="xf", name=f"xf{bt}")
                ring[bt % 2].dma_start(out=xf, in_=x[ts(bt, 128), :])
                yts = []
                for nh in range(2):
                    yt = yload.tile([128, 2048], F32)
                    ring[(bt + nh) % 2].dma_start(
                        out=yt, in_=y[ts(bt, 128), ts(nh, 2048)])
                    yts.append(yt)

                xT = xtp.tile([128, 8, 128], BF16, tag="xT", name=f"xT{bt}")
                for kb in range(8):
                    pt = psT.tile([128, 128], F32, tag="pt",
                                  name=f"ptx{bt}_{kb}")
                    nc.tensor.transpose(pt, xf[:, ts(kb, 128)], ident_f)
                    nc.scalar.copy(out=xT[:, kb, :], in_=pt)
                xts.append(xT)
                # xsum[k] += sum_b x[b, k] (from the bf16 transposed copy)
                xs_r = statsp.tile([128, 8, 1], F32, tag="xs_r", bufs=2,
                                   name=f"xs_r{bt}")
                nc.vector.reduce_sum(out=xs_r, in_=xT,
                                     axis=mybir.AxisListType.X)
                if bt == 0:
                    nc.vector.tensor_copy(out=xsum_acc, in_=xs_r[:, :, 0])
                else:
                    nc.vector.tensor_add(xsum_acc, xsum_acc, xs_r[:, :, 0])

                # pool y: gpsimd handles bt 1 and 3 (the window before it
                # must block on the AllGather), DVE the other six
                veng = nc.gpsimd if bt in (1, 3) else nc.vector
                ys = ysump.tile([128, M], F32, tag=f"ys{bt}", name=f"ys{bt}")
                for nh in range(2):
                    ytv = yts[nh].rearrange("p (q s) -> p q s", s=2)
                    u = yup.tile([128, KF], F32)
                    veng.tensor_add(u, ytv[:, :, 0], ytv[:, :, 1])
                    u2 = u.rearrange("p (i r j) -> p i r j", r=2, j=32)
                    veng.tensor_add(
                        ys[:, ts(nh, 512)].rearrange("p (i j) -> p i j", j=32),
                        u2[:, :, 0, :], u2[:, :, 1, :])
                nc.vector.reduce_sum(out=psums_all[:, bt:bt + 1], in_=ys,
                                     axis=mybir.AxisListType.X)
                ys_tiles[bt] = ys
                if bt == 3:
                    nc.gpsimd.collective_compute(
                        "AllGather", mybir.AluOpType.bypass,
                        replica_groups=[list(range(N_CORES))],
                        ins=[cc_wt_in.opt()], outs=[cc_wt_out.opt()])

            # ---- stats AllReduce: fires right after the last y pool.
            nc.vector.reduce_sum(
                out=ar_in[:, 16:17], in_=psums_all,
                axis=mybir.AxisListType.X)
            nc.vector.tensor_add(ar_in[0:1, 16:17], ar_in[0:1, 16:17],
                                 btot_s)
            nc.sync.dma_start(out=cc_s_in, in_=ar_in)
            nc.gpsimd.collective_compute(
                "AllReduce", ADD,
                replica_groups=[list(range(N_CORES))],
                ins=[cc_s_in.opt()], outs=[cc_s_out.opt()])

            # ---- gathered-weight readback: one contiguous 256 KiB DMA per
            # source rank, split across both (now idle) rings
            for c in range(8):
                ring[c % 2].dma_start(
                    out=wt_all[:, :, c, :],
                    in_=cc_wt_out[c].rearrange("k kb ml -> k (kb ml)"))

            # ---- post-collective (all on gpsimd, idle by now): per-
            # partition dot of wcol_g * xsum_g, tree-sum the 8 cols, add
            # the y+bias col, then partition_all_reduce gives every
            # partition gsum directly (reduce + broadcast in one op).
            ar_sb = statsp.tile([128, 17], F32)
            nc.sync.dma_start(out=ar_sb, in_=cc_s_out)
            nc.gpsimd.tensor_mul(ar_sb[:, 0:8], ar_sb[:, 0:8],
                                 ar_sb[:, 8:16])
            nc.gpsimd.tensor_add(ar_sb[:, 0:4], ar_sb[:, 0:4],
                                 ar_sb[:, 4:8])
            nc.gpsimd.tensor_add(ar_sb[:, 0:2], ar_sb[:, 0:2],
                                 ar_sb[:, 2:4])
            nc.gpsimd.tensor_add(ar_sb[:, 0:1], ar_sb[:, 0:1],
                                 ar_sb[:, 1:2])
            nc.gpsimd.tensor_add(ar_sb[:, 0:1], ar_sb[:, 0:1],
                                 ar_sb[:, 16:17])
            gs_pp = statsp.tile([128, 1], F32)
            nc.gpsimd.partition_all_reduce(gs_pp, ar_sb[:, 0:1],
                                           channels=128,
                                           reduce_op=bass_isa.ReduceOp.add)
            # rsb = TOT/gsum is computed on DVE, emitted mid-GEMM-loop
            # (after bt 3's epilogue) so the queue never stalls on the AR
            rinv = statsp.tile([128, 1], F32)
            rsb = statsp.tile([128, 1], F32)

            # ---- GEMM per b-tile + epilogue add into ys ----
            for bt in range(8):
                mm = [psA.tile([128, 512], F32, tag="mm", name=f"mm{bt}_{h}")
                      for h in range(2)]
                for kb in range(8):
                    for mh in range(2):
                        nc.tensor.matmul(
                            mm[mh], xts[bt][:, kb, :],
                            wt_all[:, kb, 4 * mh:4 * mh + 4, :].rearrange(
                                "k c ml -> k (c ml)"),
                            start=(kb == 0), stop=False)
                for mh in range(2):
                    nc.tensor.matmul(mm[mh], ones_row, bsum_bf[:, ts(mh, 512)],
                                     start=False, stop=True)
                    nc.vector.tensor_add(ys_tiles[bt][:, ts(mh, 512)], mm[mh],
                                         ys_tiles[bt][:, ts(mh, 512)])
                if bt == 3:
                    nc.vector.reciprocal(rinv, gs_pp)
                    nc.vector.tensor_mul(rsb, rinv, tot_pp)

            # ---- normalize + store on ACT (idle by now; DVE keeps doing
            # epilogue adds): out = pooled * (TOT/gsum), stored f16 (half
            # the store bytes; ~5e-4 extra relative error, well in budget)
            for bt in range(8):
                ot = outp.tile([128, M], F16)
                nc.scalar.mul(ot, ys_tiles[bt], rsb)
                ring[bt % 2].dma_start(out=out[ts(bt, 128), :], in_=ot)

    nc.compile()
    return nc


def _run(inputs, trace=False):
    if "nc" not in _CACHE:
        _CACHE["nc"] = build_nc()
    nc = _CACHE["nc"]
    x = np.ascontiguousarray(np.asarray(inputs["x"], dtype=np.float32))
    y = np.ascontiguousarray(np.asarray(inputs["y"], dtype=np.float32))
    w = np.ascontiguousarray(np.asarray(inputs["weight"], dtype=np.float32))
    b = np.ascontiguousarray(
        np.asarray(inputs["bias"], dtype=np.float32).reshape(1, NF))
    in_maps = [
        {"x": x[c * BL:(c + 1) * BL], "y": y[c * BL:(c + 1) * BL],
         "w": np.ascontiguousarray(w[c * WS:(c + 1) * WS]), "b": b}
        for c in range(N_CORES)
    ]
    res = run_bass_kernel_spmd(nc, in_maps, core_ids=list(range(N_CORES)),
                               trace=trace)
    full = np.concatenate(
        [np.asarray(res.results[c]["out"]).astype(np.float32)
         for c in range(N_CORES)], axis=0)
    return full.reshape(B, 1, 32, 32), res


def kernel(**inputs) -> np.ndarray:
    out, _ = _run(inputs, trace=False)
    return out
